# revision 43
# baseline (speedup 1.0000x reference)
"""AutoCorrelation (FFT cross-correlation + full-sort delay aggregation) on 8 NeuronCores.

Math (per batch b, channels c = (h,e), C = 512, L = 512):
  mv[t]   = (1/C) sum_c irfft( Q_c * conj(K_c) )[t]        (channel-mean correlation)
  rank(i) = #{m : mv[m] < mv[i]}  per batch (exact ints via is_lt+accum)
  g[b, m] = softmax(mv[b])[ time i with rank_b(i) == rank_0(m) ]
  out[b,t,c] = sum_u g[b,u] * v[b,(t+u) % L, c]            (circular correlation)

Per-core structure (4 local batches + a redundant batch-0 slot, phase-ordered
emission so each engine's in-order queue never blocks ready work):
  - chunk-interleaved first loads; all q/k/v prefetched before dependent DMAs
  - rfft of q,k as fp32r matmuls vs packed cos/sin DFT tiles (1 merged DMA/slot)
  - irfft constants (Ar/Ai/ArB0) derived on-chip from the DFT tiles via PE
    transposes + per-partition scaled copies (no 1.25MB constant load)
  - spectra products + channel reduction via DVE scalar_tensor_tensor accum
  - irfft restructured as free-1 fp32 matmuls producing mv^T [128,4] directly;
    mv row recovered with transpose-datapath identity matmuls
  - batch-0 rank row computed once (replica slot, urgent-priority chain) and
    partition-broadcast; per-slot ranks matched against it with Pool is_equal
  - circulant of g materialized as an overlapping Hankel DMA read from a
    doubled 4KB DRAM row + per-block column reversal on Pool (vs 1.5MB/slot)
  - aggregation as Toeplitz-block fp32r matmuls vs v in natural layout,
    PSUM double-banked output chains, merged 1-DMA output stores
"""

import sys
for _p in ('/opt/trn_rl_repo',):
    if _p not in sys.path:
        sys.path.insert(0, _p)

import numpy as np
from contextlib import ExitStack

import concourse.bass as bass
import concourse.bacc as bacc
import concourse.tile as tile
import concourse.mybir as mybir
from concourse.bass_utils import run_bass_kernel_spmd

F32 = mybir.dt.float32
F32R = mybir.dt.float32r
AL = mybir.AluOpType
AF = mybir.ActivationFunctionType

B, L, H, E = 32, 512, 8, 64
C = H * E          # 512 channels per batch
NCORES = 8
NB = B // NCORES   # 4 local batches per core
NSLOT = NB + 1     # + redundant batch-0 slot
BPB = L * C        # elements per batch tensor


def _consts():
    l = np.arange(L)[:, None].astype(np.float64)
    f = np.arange(257)[None, :].astype(np.float64)
    Wc = np.cos(2 * np.pi * l * f / L).astype(np.float32)            # [512, 257]
    Ws = np.sin(2 * np.pi * l * f[:, :256] / L).astype(np.float32)   # [512, 256]
    # Nyquist cos packed into Ws's all-zero f=0 column (pairs with ArB0 row 0)
    Ws[:, 0] = np.cos(np.pi * np.arange(L)).astype(np.float32)
    m = np.arange(L)[None, :].astype(np.float64)
    fc = np.arange(257)[:, None].astype(np.float64)
    wgt = np.where((fc == 0) | (fc == 256), 1.0, 2.0)
    Ar = (wgt * np.cos(2 * np.pi * fc * m / L) / (L * C)).astype(np.float32)   # [257, 512]
    fs = np.arange(256)[:, None].astype(np.float64)
    wgt_i = np.where(fs == 0, 0.0, 2.0)
    Ai = (-wgt_i * np.sin(2 * np.pi * fs * m / L) / (L * C)).astype(np.float32)  # [256, 512]
    ArB0 = Ar[0:128].copy()
    ArB0[0] = Ar[256]
    WcAll = np.zeros((128, 4 * 257), np.float32)
    WsAll = np.zeros((128, 4 * 256), np.float32)
    for lc in range(4):
        WcAll[:, 257 * lc:257 * (lc + 1)] = Wc[128 * lc:128 * (lc + 1)]
        WsAll[:, 256 * lc:256 * (lc + 1)] = Ws[128 * lc:128 * (lc + 1)]
    scl = 1.0 / (L * C)
    wgt2 = np.zeros((128, 2), np.float32)
    wgt2[:, 0] = 2 * scl; wgt2[0, 0] = scl          # Ar g0 row scales
    wgt2[:, 1] = -2 * scl; wgt2[0, 1] = 0.0         # Ai g0 row scales (kills f=0)
    return WcAll, WsAll, wgt2


_NC_CACHE = None


def _build():
    global _NC_CACHE
    if _NC_CACHE is not None:
        return _NC_CACHE
    WcAll_np, WsAll_np, wgt2_np = _consts()

    nc = bacc.Bacc("TRN2", target_bir_lowering=False, debug=False, num_devices=NCORES)
    tc = tile.TileContext(nc)

    q_all = nc.dram_tensor("q_all", [NSLOT, L, C], F32R, kind="ExternalInput")
    k_all = nc.dram_tensor("k_all", [NSLOT, L, C], F32R, kind="ExternalInput")
    v_all = nc.dram_tensor("v_all", [NB, L, C], F32R, kind="ExternalInput")
    out_all = nc.dram_tensor("out_all", [NB, L, C], F32, kind="ExternalOutput")

    Wc_d = nc.inline_tensor(WcAll_np, "Wc_d")
    Ws_d = nc.inline_tensor(WsAll_np, "Ws_d")
    wgt2_d = nc.inline_tensor(wgt2_np, "wgt2_d")

    def mio(t, s):
        """merged 3D AP into slot s of a [*, L, C] dram tensor"""
        return bass.AP(tensor=t, offset=s * BPB, ap=[[512, 128], [65536, 4], [1, 512]])

    with tc, ExitStack() as ctx:
        cpool = ctx.enter_context(tc.tile_pool(name="consts", bufs=1))
        iopool = ctx.enter_context(tc.tile_pool(name="io", bufs=1))
        wpool = ctx.enter_context(tc.tile_pool(name="work", bufs=1))
        spool = ctx.enter_context(tc.tile_pool(name="scol", bufs=1))
        pspec = ctx.enter_context(tc.tile_pool(name="pspec", bufs=1, space="PSUM"))
        psmall = ctx.enter_context(tc.tile_pool(name="psmall", bufs=1, space="PSUM"))
        dpool = ctx.enter_context(tc.tile_pool(name="dscratch", bufs=1, space="DRAM"))

        # ---- constants + first-slot loads, chunk-interleaved so the first
        # FFT matmul can start after ~2 chunk DMAs instead of ~8us of loads
        Wc_t = [cpool.tile([128, 257], F32R, name=f"Wc_t{lc}") for lc in range(4)]
        Ws_t = [cpool.tile([128, 256], F32R, name=f"Ws_t{lc}") for lc in range(4)]
        q0_t = [iopool.tile([128, 512], F32R, name=f"q0_{lc}", tag="q0", bufs=4) for lc in range(4)]
        k0_t = [iopool.tile([128, 512], F32R, name=f"k0_{lc}", tag="k0", bufs=4) for lc in range(4)]
        for lc in range(4):
            nc.sync.dma_start(Wc_t[lc][:], bass.AP(tensor=Wc_d, offset=257 * lc,
                                                   ap=[[1028, 128], [1, 257]]).bitcast(F32R))
            nc.sync.dma_start(Ws_t[lc][:], bass.AP(tensor=Ws_d, offset=256 * lc,
                                                   ap=[[1024, 128], [1, 256]]).bitcast(F32R))
            nc.sync.dma_start(q0_t[lc][:], bass.AP(tensor=q_all, offset=4 * BPB + 65536 * lc,
                                                   ap=[[512, 128], [1, 512]]).bitcast(F32R))
            nc.sync.dma_start(k0_t[lc][:], bass.AP(tensor=k_all, offset=4 * BPB + 65536 * lc,
                                                   ap=[[512, 128], [1, 512]]).bitcast(F32R))
        RT = cpool.tile([128, 2560], F32, name="RT")
        wgt2_t = cpool.tile([128, 2], F32, name="wgt2_t")
        nc.sync.dma_start(wgt2_t[:], wgt2_d.ap())
        idI = cpool.tile([128, 128], F32, name="idI")
        iotc = cpool.tile([128, 1], F32, name="iotc")
        iotr = cpool.tile([128, 128], F32, name="iotr")
        nc.gpsimd.iota(iotc[:], [[0, 1]], base=0, channel_multiplier=1,
                       allow_small_or_imprecise_dtypes=True)
        nc.gpsimd.iota(iotr[:], [[1, 128]], base=0, channel_multiplier=0,
                       allow_small_or_imprecise_dtypes=True)
        nc.vector.tensor_scalar(idI[:], iotr[:], iotc[:], None, AL.is_equal)
        # ---- derive RT = [Ar0|Ar1|Ai0|Ai1|ArB0] on-chip from Wc/Ws ----
        SCL = 1.0 / (L * C)
        for g in range(2):
            for (base, Wt) in ((0, Wc_t), (1024, Ws_t)):
                rp = psmall.tile([128, 512], F32, name=f"rp_{base}_{g}", tag="outp", bufs=1)
                for lc in range(4):
                    nc.tensor.transpose(rp[:, 128 * lc:128 * (lc + 1)],
                                        Wt[lc][:, 128 * g:128 * (g + 1)].bitcast(F32), idI[:])
                col = base + 512 * g
                if base == 0 and g == 0:
                    nc.scalar.activation(RT[:, col:col + 512], rp[:], AF.Copy, bias=0.0, scale=wgt2_t[:, 0:1])
                elif base == 0:
                    nc.scalar.activation(RT[:, col:col + 512], rp[:], AF.Copy, bias=0.0, scale=2.0 * SCL)
                elif g == 0:
                    nc.scalar.activation(RT[:, col:col + 512], rp[:], AF.Copy, bias=0.0, scale=wgt2_t[:, 1:2])
                else:
                    nc.scalar.activation(RT[:, col:col + 512], rp[:], AF.Copy, bias=0.0, scale=-2.0 * SCL)
        # ArB0 = Ar0 with row 0 replaced by the Nyquist cos row / (L*C)
        nc.scalar.copy(RT[:, 2048:2560], RT[:, 0:512])
        nyq_ps = psmall.tile([1, 512], F32, name="nyq_ps", tag="psm", bufs=1)
        for lc in range(4):
            nc.tensor.transpose(nyq_ps[0:1, 128 * lc:128 * (lc + 1)],
                                Ws_t[lc][:, 0:1].bitcast(F32), idI[:])
        nc.scalar.activation(RT[0:1, 2048:2560], nyq_ps[:], AF.Copy, bias=0.0, scale=SCL)

        n2bB = cpool.tile([128, 512], F32, name="n2bB")
        encBlt = cpool.tile([128, 512], F32, name="encBlt")
        nc.gpsimd.iota(encBlt[:], [[-1, 512]], base=511, channel_multiplier=0,
                       allow_small_or_imprecise_dtypes=True)
        encCol = cpool.tile([128, 4], F32, name="encCol")
        one_t = cpool.tile([1, 1], F32, name="one_t")
        nc.vector.memset(one_t[:], 1.0)
        nc.gpsimd.iota(encCol[:], [[-128, 4]], base=511, channel_multiplier=-1,
                       allow_small_or_imprecise_dtypes=True)

        def P(bi):
            bi.ins.bass_priority = -50
            return bi

        def P2(bi):
            bi.ins.bass_priority = -100
            return bi

        # R column bases for the 5 scols: Ar_g0, Ar_g1, Ai_g0, Ai_g1, ArB0
        RB_AR = (0, 512)
        RB_AI = (1024, 1536)
        RB_ARB0 = 2048

        # ---------- per-slot stage A: mvT + mv row + broadcast ----------
        def stage_mv(s, q_srcs, k_srcs, PP=None):
            PP = PP or P
            scols = []   # (scol, R column base) for the irfft matmuls
            for g in range(2):
                sq = pspec.tile([128, 1024], F32, name=f"specq_s{s}g{g}", tag="spec", bufs=3)
                sk = pspec.tile([128, 1024], F32, name=f"speck_s{s}g{g}", tag="spec", bufs=3)
                for (spec, srcs) in ((sq, q_srcs), (sk, k_srcs)):
                    for lc in range(4):
                        nc.tensor.matmul(spec[:, 0:512],
                                         Wc_t[lc][:, 128 * g: 128 * (g + 1)],
                                         srcs(lc), start=(lc == 0), stop=(lc == 3))
                    for lc in range(4):
                        nc.tensor.matmul(spec[:, 512:1024],
                                         Ws_t[lc][:, 128 * g: 128 * (g + 1)],
                                         srcs(lc), start=(lc == 0), stop=(lc == 3))
                sk_sb = wpool.tile([128, 1024], F32, name=f"sksb_s{s}g{g}", tag="sksb", bufs=2)
                PP(nc.scalar.copy(sk_sb[:], sk[:]))
                scr = wpool.tile([128, 1024], F32R, name=f"sttscr_s{s}g{g}", tag="sttscr", bufs=1)
                si1 = spool.tile([128, 1], F32, name=f"si1_s{s}g{g}", tag=f"si1{g}", bufs=2)
                si2 = spool.tile([128, 1], F32, name=f"si2_s{s}g{g}", tag=f"si2{g}", bufs=2)
                si = spool.tile([128, 1], F32, name=f"si_s{s}g{g}", tag=f"si{g}", bufs=2)
                if g == 0:
                    # cos/sin halves reduced separately: the sin-half partition 0
                    # carries the Nyquist product and pairs with ArB0's row 0.
                    srA = spool.tile([128, 1], F32, name=f"srA_s{s}", tag="srA", bufs=2)
                    srB = spool.tile([128, 1], F32, name=f"srB_s{s}", tag="srB", bufs=2)
                    PP(nc.vector.scalar_tensor_tensor(scr[:, 0:512], sq[:, 0:512], 1.0, sk_sb[:, 0:512], AL.mult, AL.mult, accum_out=srA[:]))
                    PP(nc.vector.scalar_tensor_tensor(scr[:, 512:1024], sq[:, 512:1024], 1.0, sk_sb[:, 512:1024], AL.mult, AL.mult, accum_out=srB[:]))
                    scols.append((srA, RB_AR[0]))
                    scols.append((srB, RB_ARB0))
                else:
                    sr = spool.tile([128, 1], F32, name=f"sr_s{s}g{g}", tag=f"sr{g}", bufs=2)
                    PP(nc.vector.scalar_tensor_tensor(scr[:], sq[:, 0:1024], 1.0, sk_sb[:, 0:1024], AL.mult, AL.mult, accum_out=sr[:]))
                    scols.append((sr, RB_AR[1]))
                # S_i = sum_c Qr*Ks - Qs*Kr  (f=0 garbage killed by Ai's zero row)
                PP(nc.vector.scalar_tensor_tensor(scr[:, 0:512], sq[:, 0:512], 1.0, sk_sb[:, 512:1024], AL.mult, AL.mult, accum_out=si1[:]))
                PP(nc.vector.scalar_tensor_tensor(scr[:, 0:512], sq[:, 512:1024], 1.0, sk_sb[:, 0:512], AL.mult, AL.mult, accum_out=si2[:]))
                PP(nc.vector.tensor_sub(si[:], si1[:], si2[:]))
                scols.append((si, RB_AI[g]))

            # irfft: mvT[t mod 128, t div 128] directly via free-1 fp32 matmuls
            mvT_ps = psmall.tile([128, 4], F32, name=f"mvT_s{s}", tag="psm", bufs=1)
            n = len(scols)
            for tt in range(4):
                for i, (scol, base) in enumerate(scols):
                    PP(nc.tensor.matmul(mvT_ps[:, tt:tt + 1],
                                        RT[:, base + 128 * tt: base + 128 * (tt + 1)],
                                        scol[:], start=(i == 0), stop=(i == n - 1)))
            mvT_sb = wpool.tile([128, 4], F32, name=f"mvTsb_s{s}", tag="mvTsb", bufs=3)
            PP(nc.scalar.copy(mvT_sb[:], mvT_ps[:]))
            # mv row via identity matmuls (acts as 128x1 -> 1x128 transpose)
            mv_ps = psmall.tile([1, 512], F32, name=f"mvps_s{s}", tag="psm", bufs=1)
            for tt in range(4):
                PP(nc.tensor.matmul(mv_ps[0:1, 128 * tt:128 * (tt + 1)],
                                    mvT_sb[:, tt:tt + 1], idI[:], start=True, stop=True,
                                    is_transpose=True))
            mv_sb = wpool.tile([1, 512], F32, name=f"mvsb_s{s}", tag="mvsb", bufs=3)
            PP(nc.scalar.copy(mv_sb[:], mv_ps[:]))
            mvB = wpool.tile([128, 512], F32, name=f"mvB_s{s}", tag="mvB", bufs=2)
            PP(nc.gpsimd.partition_broadcast(mvB[:], mv_sb[:]))
            return mvT_sb, mv_sb, mvB

        def ranks(s, mvT_sb, mvB, PP=None):
            PP = PP or P
            # rs[p, j] = #{m: mv[m] < mv[128j+p]} (exact ints); desc-rank
            # indexing everywhere via the 511-m comparison rows
            rs = wpool.tile([128, 4], F32, name=f"rs_{s}", tag="rs", bufs=3)
            for j in range(4):
                scr = wpool.tile([128, 512], F32, name=f"rscr_{s}_{j}", tag="rscr", bufs=1)
                PP(nc.vector.tensor_scalar(scr[:], mvB[:], mvT_sb[:, j:j + 1], 0.0,
                                           AL.is_lt, AL.add, accum_out=rs[:, j:j + 1]))
            return rs

        # ---------- slot 4 (batch-0 replica) first: seeds the rank row ----------
        mvT_sb4, mv_sb4, mvB4 = stage_mv(4, lambda lc: q0_t[lc][:], lambda lc: k0_t[lc][:], PP=P2)

        # ---------- prefetch local-slot inputs (v after all q/k) ----------
        qts, kts, vts = [], [], []
        for s in range(NB):
            qt = iopool.tile([128, 2048], F32R, name=f"q_s{s}", tag="qt", bufs=3)
            nc.sync.dma_start(qt[:], mio(q_all, s).bitcast(F32R))
            qts.append(qt)
            kt = iopool.tile([128, 2048], F32R, name=f"k_s{s}", tag="kt", bufs=3)
            nc.sync.dma_start(kt[:], mio(k_all, s).bitcast(F32R))
            kts.append(kt)
        for s in range(0, NB):
            vt = iopool.tile([128, 2048], F32R, name=f"v_s{s}", tag="vt", bufs=4)
            nc.sync.dma_start(vt[:], mio(v_all, s).bitcast(F32R))
            vts.append(vt)

        rs_4 = ranks(4, mvT_sb4, mvB4, PP=P2)
        n2b_ps = psmall.tile([1, 512], F32, name="n2b_ps", tag="psm", bufs=1)
        for tt in range(4):
            P2(nc.tensor.matmul(n2b_ps[0:1, 128 * tt:128 * (tt + 1)],
                                rs_4[:, tt:tt + 1], idI[:], start=True, stop=True,
                                is_transpose=True))
        n2b_sb = wpool.tile([1, 512], F32, name="n2b_sb", bufs=1)
        P2(nc.scalar.copy(n2b_sb[:], n2b_ps[:]))
        P2(nc.gpsimd.partition_broadcast(n2bB[:], n2b_sb[:]))

        # ---------- local slots: phase A (FFT/mv/ranks) ----------
        mvs = []
        for s in range(0, NB):
            qt, kt = qts[s], kts[s]
            mvT_sb, mv_sb, mvB = stage_mv(s, lambda lc, qt=qt: qt[:, 512 * lc:512 * (lc + 1)],
                                          lambda lc, kt=kt: kt[:, 512 * lc:512 * (lc + 1)])
            rs = ranks(s, mvT_sb, mvB)
            mvs.append((mvT_sb, mv_sb, mvB, rs))

        # ---------- phase B for all slots (rank-match, g row, circulant fetch) ----------
        bres = []
        for s in range(NB):
            mvT_sb, mv_sb, mvB, rs = mvs[s]
            # softmax pieces
            expscr = wpool.tile([1, 512], F32, name=f"expscr_{s}", tag="expscr", bufs=1)
            z_sb = wpool.tile([1, 1], F32, name=f"z_{s}", tag="z", bufs=2)
            P(nc.scalar.activation(expscr[:], mv_sb[:], AF.Exp, accum_out=z_sb[:]))
            rz = wpool.tile([1, 1], F32, name=f"rz_{s}", tag="rz", bufs=2)
            P(nc.vector.reciprocal(rz[:], z_sb[:]))
            smc = wpool.tile([128, 4], F32, name=f"smc_{s}", tag="smc", bufs=2)
            P(nc.scalar.activation(smc[:], mvT_sb[:], AF.Exp))
            smcr = wpool.tile([128, 4], F32R, name=f"smcr_{s}", tag="smcr", bufs=2)
            P(nc.vector.tensor_copy(smcr[:], smc[:]))

            # g row via rank match vs the batch-0 row (both raw is_lt counts)
            g_ps = psmall.tile([1, 512], F32, name=f"gps_{s}", tag="outp", bufs=1)
            for j in range(4):
                wt = wpool.tile([128, 512], F32R, name=f"wt_{s}_{j}", tag=f"wt{j}", bufs=1)
                P(nc.gpsimd.tensor_scalar(wt[:], n2bB[:], rs[:, j:j + 1], None, AL.is_equal))
                P(nc.tensor.matmul(g_ps[:], smcr[:, j:j + 1], wt[:], start=(j == 0), stop=(j == 3)))
            gn = wpool.tile([1, 1024], F32, name=f"gn_{s}", tag="gn", bufs=2)
            PB = P2 if s == NB - 1 else P
            PB(nc.scalar.activation(gn[0:1, 0:512], g_ps[:], AF.Copy, bias=0.0, scale=rz[:]))
            PB(nc.scalar.activation(gn[0:1, 512:1024], g_ps[:], AF.Copy, bias=0.0, scale=rz[:]))

            # ---------- stage C: circulant via doubled row + Hankel + reversal ----------
            gbuf = dpool.tile([1, 1024], F32, name=f"gbuf_{s}", tag="gbuf", bufs=4)
            gb = gbuf[:].tensor
            PB(nc.sync.dma_start(bass.AP(tensor=gb, offset=gbuf[:].offset, ap=[[1024, 1], [1, 1024]]), gn[:]))
            Hk = iopool.tile([128, 512], F32, name=f"H_{s}", tag="Hk", bufs=2)
            PB(nc.sync.dma_start(Hk[:], bass.AP(tensor=gb, offset=gbuf[:].offset + 385, ap=[[1, 128], [1, 512]])))
            Tt = wpool.tile([128, 512], F32R, name=f"T_{s}", tag="Tt", bufs=4)
            hap = Hk[:]
            rev = bass.AP(tensor=hap.tensor, offset=hap.offset + 127,
                          ap=[[hap.ap[0][0], 128], [128, 4], [-1, 128]])
            PB(nc.gpsimd.tensor_copy(Tt[:], rev))
            bres.append(Tt)

        # ---------- phase C per slot: Toeplitz-block aggregation ----------
        for s in range(NB):
            Tt = bres[s]
            vt = vts[s]
            oAll = wpool.tile([128, 2048], F32, name=f"oAll_{s}", tag="oAll", bufs=2)
            for tt in range(4):
                o_ps = psmall.tile([128, 512], F32, name=f"ops_{s}_{tt}",
                                   tag=("outp" if tt % 2 == 0 else "psm"), bufs=1)
                for ss in range(4):
                    d = (ss - tt) % 4
                    nc.tensor.matmul(o_ps[:], Tt[:, 128 * d:128 * (d + 1)],
                                     vt[:, 512 * ss:512 * (ss + 1)],
                                     start=(ss == 0), stop=(ss == 3))
                osb = oAll[:, 512 * tt:512 * (tt + 1)]
                if tt % 2 == 0:
                    P(nc.scalar.copy(osb, o_ps[:]))
                else:
                    P(nc.vector.tensor_copy(osb, o_ps[:]))
                # per-chunk store: the last chunk's 256KB write is all that
                # remains after the final copy, instead of a 1MB slot write
                P(nc.sync.dma_start(bass.AP(tensor=out_all, offset=s * BPB + 65536 * tt,
                                            ap=[[512, 128], [1, 512]]), osb))

    nc.compile()
    _NC_CACHE = nc
    return nc


def kernel(queries, keys, values):
    q = np.ascontiguousarray(queries, dtype=np.float32).reshape(B, L, C)
    k = np.ascontiguousarray(keys, dtype=np.float32).reshape(B, L, C)
    v = np.ascontiguousarray(values, dtype=np.float32).reshape(B, L, C)
    nc = _build()
    in_maps = []
    for c in range(NCORES):
        sl = slice(NB * c, NB * (c + 1))
        in_maps.append({
            "q_all": np.concatenate([q[sl], q[0:1]], axis=0),
            "k_all": np.concatenate([k[sl], k[0:1]], axis=0),
            "v_all": v[sl],
        })
    res = run_bass_kernel_spmd(nc, in_maps, core_ids=list(range(NCORES)))
    out = np.concatenate([res.results[c]["out_all"] for c in range(NCORES)], axis=0)
    return out.reshape(B, L, H, E)


if __name__ == "__main__":
    rng = np.random.default_rng(0)
    qq = rng.standard_normal((B, L, H, E)).astype(np.float32)
    kk = rng.standard_normal((B, L, H, E)).astype(np.float32)
    vv = rng.standard_normal((B, L, H, E)).astype(np.float32)
    o = kernel(queries=qq, keys=kk, values=vv)
    print(o.shape, o.dtype, np.abs(o).max())


# revision 44
# speedup vs baseline: 1.0067x; 1.0067x over previous
"""AutoCorrelation (FFT cross-correlation + full-sort delay aggregation) on 8 NeuronCores.

Math (per batch b, channels c = (h,e), C = 512, L = 512):
  mv[t]   = (1/C) sum_c irfft( Q_c * conj(K_c) )[t]        (channel-mean correlation)
  rank(i) = #{m : mv[m] < mv[i]}  per batch (exact ints via is_lt+accum)
  g[b, m] = softmax(mv[b])[ time i with rank_b(i) == rank_0(m) ]
  out[b,t,c] = sum_u g[b,u] * v[b,(t+u) % L, c]            (circular correlation)

Per-core structure (4 local batches + a redundant batch-0 slot, phase-ordered
emission so each engine's in-order queue never blocks ready work):
  - chunk-interleaved first loads; all q/k/v prefetched before dependent DMAs
  - rfft of q,k as fp32r matmuls vs packed cos/sin DFT tiles (1 merged DMA/slot)
  - irfft constants (Ar/Ai/ArB0) derived on-chip from the DFT tiles via PE
    transposes + per-partition scaled copies (no 1.25MB constant load)
  - spectra products + channel reduction via DVE scalar_tensor_tensor accum
  - irfft restructured as free-1 fp32 matmuls producing mv^T [128,4] directly;
    mv row recovered with transpose-datapath identity matmuls
  - batch-0 rank row computed once (replica slot, urgent-priority chain) and
    partition-broadcast; per-slot ranks matched against it with Pool is_equal
  - circulant of g materialized as an overlapping Hankel DMA read from a
    doubled 4KB DRAM row + per-block column reversal on Pool (vs 1.5MB/slot)
  - aggregation as Toeplitz-block fp32r matmuls vs v in natural layout,
    PSUM double-banked output chains, merged 1-DMA output stores
"""

import sys
for _p in ('/opt/trn_rl_repo',):
    if _p not in sys.path:
        sys.path.insert(0, _p)

import numpy as np
from contextlib import ExitStack

import concourse.bass as bass
import concourse.bacc as bacc
import concourse.tile as tile
import concourse.mybir as mybir
from concourse.bass_utils import run_bass_kernel_spmd

F32 = mybir.dt.float32
F32R = mybir.dt.float32r
AL = mybir.AluOpType
AF = mybir.ActivationFunctionType

B, L, H, E = 32, 512, 8, 64
C = H * E          # 512 channels per batch
NCORES = 8
NB = B // NCORES   # 4 local batches per core
NSLOT = NB + 1     # + redundant batch-0 slot
BPB = L * C        # elements per batch tensor


def _consts():
    l = np.arange(L)[:, None].astype(np.float64)
    f = np.arange(257)[None, :].astype(np.float64)
    Wc = np.cos(2 * np.pi * l * f / L).astype(np.float32)            # [512, 257]
    Ws = np.sin(2 * np.pi * l * f[:, :256] / L).astype(np.float32)   # [512, 256]
    # Nyquist cos packed into Ws's all-zero f=0 column (pairs with ArB0 row 0)
    Ws[:, 0] = np.cos(np.pi * np.arange(L)).astype(np.float32)
    m = np.arange(L)[None, :].astype(np.float64)
    fc = np.arange(257)[:, None].astype(np.float64)
    wgt = np.where((fc == 0) | (fc == 256), 1.0, 2.0)
    Ar = (wgt * np.cos(2 * np.pi * fc * m / L) / (L * C)).astype(np.float32)   # [257, 512]
    fs = np.arange(256)[:, None].astype(np.float64)
    wgt_i = np.where(fs == 0, 0.0, 2.0)
    Ai = (-wgt_i * np.sin(2 * np.pi * fs * m / L) / (L * C)).astype(np.float32)  # [256, 512]
    ArB0 = Ar[0:128].copy()
    ArB0[0] = Ar[256]
    WcAll = np.zeros((128, 4 * 257), np.float32)
    WsAll = np.zeros((128, 4 * 256), np.float32)
    for lc in range(4):
        WcAll[:, 257 * lc:257 * (lc + 1)] = Wc[128 * lc:128 * (lc + 1)]
        WsAll[:, 256 * lc:256 * (lc + 1)] = Ws[128 * lc:128 * (lc + 1)]
    scl = 1.0 / (L * C)
    wgt2 = np.zeros((128, 2), np.float32)
    wgt2[:, 0] = 2 * scl; wgt2[0, 0] = scl          # Ar g0 row scales
    wgt2[:, 1] = -2 * scl; wgt2[0, 1] = 0.0         # Ai g0 row scales (kills f=0)
    return WcAll, WsAll, wgt2


_NC_CACHE = None


def _build():
    global _NC_CACHE
    if _NC_CACHE is not None:
        return _NC_CACHE
    WcAll_np, WsAll_np, wgt2_np = _consts()

    nc = bacc.Bacc("TRN2", target_bir_lowering=False, debug=False, num_devices=NCORES)
    tc = tile.TileContext(nc)

    q_all = nc.dram_tensor("q_all", [NSLOT, L, C], F32R, kind="ExternalInput")
    k_all = nc.dram_tensor("k_all", [NSLOT, L, C], F32R, kind="ExternalInput")
    v_all = nc.dram_tensor("v_all", [NB, L, C], F32R, kind="ExternalInput")
    out_all = nc.dram_tensor("out_all", [NB, L, C], F32, kind="ExternalOutput")

    Wc_d = nc.inline_tensor(WcAll_np, "Wc_d")
    Ws_d = nc.inline_tensor(WsAll_np, "Ws_d")
    wgt2_d = nc.inline_tensor(wgt2_np, "wgt2_d")

    def mio(t, s):
        """merged 3D AP into slot s of a [*, L, C] dram tensor"""
        return bass.AP(tensor=t, offset=s * BPB, ap=[[512, 128], [65536, 4], [1, 512]])

    with tc, ExitStack() as ctx:
        cpool = ctx.enter_context(tc.tile_pool(name="consts", bufs=1))
        iopool = ctx.enter_context(tc.tile_pool(name="io", bufs=1))
        wpool = ctx.enter_context(tc.tile_pool(name="work", bufs=1))
        spool = ctx.enter_context(tc.tile_pool(name="scol", bufs=1))
        pspec = ctx.enter_context(tc.tile_pool(name="pspec", bufs=1, space="PSUM"))
        psmall = ctx.enter_context(tc.tile_pool(name="psmall", bufs=1, space="PSUM"))
        dpool = ctx.enter_context(tc.tile_pool(name="dscratch", bufs=1, space="DRAM"))

        # ---- constants + first-slot loads, chunk-interleaved so the first
        # FFT matmul can start after ~2 chunk DMAs instead of ~8us of loads
        Wc_t = [cpool.tile([128, 257], F32R, name=f"Wc_t{lc}") for lc in range(4)]
        Ws_t = [cpool.tile([128, 256], F32R, name=f"Ws_t{lc}") for lc in range(4)]
        q0_t = [iopool.tile([128, 512], F32R, name=f"q0_{lc}", tag="q0", bufs=4) for lc in range(4)]
        k0_t = [iopool.tile([128, 512], F32R, name=f"k0_{lc}", tag="k0", bufs=4) for lc in range(4)]
        for lc in range(4):
            nc.sync.dma_start(Wc_t[lc][:], bass.AP(tensor=Wc_d, offset=257 * lc,
                                                   ap=[[1028, 128], [1, 257]]).bitcast(F32R))
            nc.sync.dma_start(Ws_t[lc][:], bass.AP(tensor=Ws_d, offset=256 * lc,
                                                   ap=[[1024, 128], [1, 256]]).bitcast(F32R))
            nc.sync.dma_start(q0_t[lc][:], bass.AP(tensor=q_all, offset=4 * BPB + 65536 * lc,
                                                   ap=[[512, 128], [1, 512]]).bitcast(F32R))
            nc.sync.dma_start(k0_t[lc][:], bass.AP(tensor=k_all, offset=4 * BPB + 65536 * lc,
                                                   ap=[[512, 128], [1, 512]]).bitcast(F32R))
        RT = cpool.tile([128, 2560], F32, name="RT")
        wgt2_t = cpool.tile([128, 2], F32, name="wgt2_t")
        nc.sync.dma_start(wgt2_t[:], wgt2_d.ap())
        idI = cpool.tile([128, 128], F32, name="idI")
        iotc = cpool.tile([128, 1], F32, name="iotc")
        iotr = cpool.tile([128, 128], F32, name="iotr")
        nc.gpsimd.iota(iotc[:], [[0, 1]], base=0, channel_multiplier=1,
                       allow_small_or_imprecise_dtypes=True)
        nc.gpsimd.iota(iotr[:], [[1, 128]], base=0, channel_multiplier=0,
                       allow_small_or_imprecise_dtypes=True)
        nc.vector.tensor_scalar(idI[:], iotr[:], iotc[:], None, AL.is_equal)
        # ---- derive RT = [Ar0|Ar1|Ai0|Ai1|ArB0] on-chip from Wc/Ws ----
        SCL = 1.0 / (L * C)
        for g in range(2):
            for (base, Wt) in ((0, Wc_t), (1024, Ws_t)):
                rp = psmall.tile([128, 512], F32, name=f"rp_{base}_{g}", tag="outp", bufs=1)
                for lc in range(4):
                    nc.tensor.transpose(rp[:, 128 * lc:128 * (lc + 1)],
                                        Wt[lc][:, 128 * g:128 * (g + 1)].bitcast(F32), idI[:])
                col = base + 512 * g
                if base == 0 and g == 0:
                    nc.scalar.activation(RT[:, col:col + 512], rp[:], AF.Copy, bias=0.0, scale=wgt2_t[:, 0:1])
                elif base == 0:
                    nc.scalar.activation(RT[:, col:col + 512], rp[:], AF.Copy, bias=0.0, scale=2.0 * SCL)
                elif g == 0:
                    nc.scalar.activation(RT[:, col:col + 512], rp[:], AF.Copy, bias=0.0, scale=wgt2_t[:, 1:2])
                else:
                    nc.scalar.activation(RT[:, col:col + 512], rp[:], AF.Copy, bias=0.0, scale=-2.0 * SCL)
        # ArB0 = Ar0 with row 0 replaced by the Nyquist cos row / (L*C)
        nc.scalar.copy(RT[:, 2048:2560], RT[:, 0:512])
        nyq_ps = psmall.tile([1, 512], F32, name="nyq_ps", tag="psm", bufs=1)
        for lc in range(4):
            nc.tensor.transpose(nyq_ps[0:1, 128 * lc:128 * (lc + 1)],
                                Ws_t[lc][:, 0:1].bitcast(F32), idI[:])
        nc.scalar.activation(RT[0:1, 2048:2560], nyq_ps[:], AF.Copy, bias=0.0, scale=SCL)

        n2bB = cpool.tile([128, 512], F32, name="n2bB")
        encBlt = cpool.tile([128, 512], F32, name="encBlt")
        nc.gpsimd.iota(encBlt[:], [[-1, 512]], base=511, channel_multiplier=0,
                       allow_small_or_imprecise_dtypes=True)
        encCol = cpool.tile([128, 4], F32, name="encCol")
        one_t = cpool.tile([1, 1], F32, name="one_t")
        nc.vector.memset(one_t[:], 1.0)
        nc.gpsimd.iota(encCol[:], [[-128, 4]], base=511, channel_multiplier=-1,
                       allow_small_or_imprecise_dtypes=True)

        def P(bi):
            bi.ins.bass_priority = -50
            return bi

        def P2(bi):
            bi.ins.bass_priority = -100
            return bi

        # R column bases for the 5 scols: Ar_g0, Ar_g1, Ai_g0, Ai_g1, ArB0
        RB_AR = (0, 512)
        RB_AI = (1024, 1536)
        RB_ARB0 = 2048

        # ---------- per-slot stage A: mvT + mv row + broadcast ----------
        def stage_mv(s, q_srcs, k_srcs, PP=None):
            PP = PP or P
            scols = []   # (scol, R column base) for the irfft matmuls
            for g in range(2):
                sq = pspec.tile([128, 1024], F32, name=f"specq_s{s}g{g}", tag="spec", bufs=3)
                sk = pspec.tile([128, 1024], F32, name=f"speck_s{s}g{g}", tag="spec", bufs=3)
                for (spec, srcs) in ((sq, q_srcs), (sk, k_srcs)):
                    for lc in range(4):
                        nc.tensor.matmul(spec[:, 0:512],
                                         Wc_t[lc][:, 128 * g: 128 * (g + 1)],
                                         srcs(lc), start=(lc == 0), stop=(lc == 3))
                    for lc in range(4):
                        nc.tensor.matmul(spec[:, 512:1024],
                                         Ws_t[lc][:, 128 * g: 128 * (g + 1)],
                                         srcs(lc), start=(lc == 0), stop=(lc == 3))
                sk_sb = wpool.tile([128, 1024], F32, name=f"sksb_s{s}g{g}", tag="sksb", bufs=2)
                PP(nc.scalar.copy(sk_sb[:], sk[:]))
                scr = wpool.tile([128, 1024], F32R, name=f"sttscr_s{s}g{g}", tag="sttscr", bufs=1)
                si1 = spool.tile([128, 1], F32, name=f"si1_s{s}g{g}", tag=f"si1{g}", bufs=2)
                si2 = spool.tile([128, 1], F32, name=f"si2_s{s}g{g}", tag=f"si2{g}", bufs=2)
                si = spool.tile([128, 1], F32, name=f"si_s{s}g{g}", tag=f"si{g}", bufs=2)
                if g == 0:
                    # cos/sin halves reduced separately: the sin-half partition 0
                    # carries the Nyquist product and pairs with ArB0's row 0.
                    srA = spool.tile([128, 1], F32, name=f"srA_s{s}", tag="srA", bufs=2)
                    srB = spool.tile([128, 1], F32, name=f"srB_s{s}", tag="srB", bufs=2)
                    PP(nc.vector.scalar_tensor_tensor(scr[:, 0:512], sq[:, 0:512], 1.0, sk_sb[:, 0:512], AL.mult, AL.mult, accum_out=srA[:]))
                    PP(nc.vector.scalar_tensor_tensor(scr[:, 512:1024], sq[:, 512:1024], 1.0, sk_sb[:, 512:1024], AL.mult, AL.mult, accum_out=srB[:]))
                    scols.append((srA, RB_AR[0]))
                    scols.append((srB, RB_ARB0))
                else:
                    sr = spool.tile([128, 1], F32, name=f"sr_s{s}g{g}", tag=f"sr{g}", bufs=2)
                    PP(nc.vector.scalar_tensor_tensor(scr[:], sq[:, 0:1024], 1.0, sk_sb[:, 0:1024], AL.mult, AL.mult, accum_out=sr[:]))
                    scols.append((sr, RB_AR[1]))
                # S_i = sum_c Qr*Ks - Qs*Kr  (f=0 garbage killed by Ai's zero row)
                PP(nc.vector.scalar_tensor_tensor(scr[:, 0:512], sq[:, 0:512], 1.0, sk_sb[:, 512:1024], AL.mult, AL.mult, accum_out=si1[:]))
                PP(nc.vector.scalar_tensor_tensor(scr[:, 0:512], sq[:, 512:1024], 1.0, sk_sb[:, 0:512], AL.mult, AL.mult, accum_out=si2[:]))
                PP(nc.vector.tensor_sub(si[:], si1[:], si2[:]))
                scols.append((si, RB_AI[g]))

            # irfft: mvT[t mod 128, t div 128] directly via free-1 fp32 matmuls
            mvT_ps = psmall.tile([128, 4], F32, name=f"mvT_s{s}", tag="psm", bufs=1)
            n = len(scols)
            for tt in range(4):
                for i, (scol, base) in enumerate(scols):
                    PP(nc.tensor.matmul(mvT_ps[:, tt:tt + 1],
                                        RT[:, base + 128 * tt: base + 128 * (tt + 1)],
                                        scol[:], start=(i == 0), stop=(i == n - 1)))
            mvT_sb = wpool.tile([128, 4], F32, name=f"mvTsb_s{s}", tag="mvTsb", bufs=3)
            PP(nc.scalar.copy(mvT_sb[:], mvT_ps[:]))
            # mv row via identity matmuls (acts as 128x1 -> 1x128 transpose)
            mv_ps = psmall.tile([1, 512], F32, name=f"mvps_s{s}", tag="psm", bufs=1)
            for tt in range(4):
                PP(nc.tensor.matmul(mv_ps[0:1, 128 * tt:128 * (tt + 1)],
                                    mvT_sb[:, tt:tt + 1], idI[:], start=True, stop=True,
                                    is_transpose=True))
            mv_sb = wpool.tile([1, 512], F32, name=f"mvsb_s{s}", tag="mvsb", bufs=3)
            PP(nc.scalar.copy(mv_sb[:], mv_ps[:]))
            mvB = wpool.tile([128, 512], F32, name=f"mvB_s{s}", tag="mvB", bufs=2)
            PP(nc.gpsimd.partition_broadcast(mvB[:], mv_sb[:]))
            return mvT_sb, mv_sb, mvB

        def ranks(s, mvT_sb, mvB, PP=None):
            PP = PP or P
            # rs[p, j] = #{m: mv[m] < mv[128j+p]} (exact ints); desc-rank
            # indexing everywhere via the 511-m comparison rows
            rs = wpool.tile([128, 4], F32, name=f"rs_{s}", tag="rs", bufs=3)
            for j in range(4):
                scr = wpool.tile([128, 512], F32, name=f"rscr_{s}_{j}", tag="rscr", bufs=1)
                PP(nc.vector.tensor_scalar(scr[:], mvB[:], mvT_sb[:, j:j + 1], 0.0,
                                           AL.is_lt, AL.add, accum_out=rs[:, j:j + 1]))
            return rs

        # ---------- slot 4 (batch-0 replica) first: seeds the rank row ----------
        mvT_sb4, mv_sb4, mvB4 = stage_mv(4, lambda lc: q0_t[lc][:], lambda lc: k0_t[lc][:], PP=P2)

        # ---------- prefetch local-slot inputs (v after all q/k) ----------
        qts, kts, vts = [], [], []
        for s in range(NB):
            qt = iopool.tile([128, 2048], F32R, name=f"q_s{s}", tag="qt", bufs=3)
            nc.sync.dma_start(qt[:], mio(q_all, s).bitcast(F32R))
            qts.append(qt)
            kt = iopool.tile([128, 2048], F32R, name=f"k_s{s}", tag="kt", bufs=3)
            nc.sync.dma_start(kt[:], mio(k_all, s).bitcast(F32R))
            kts.append(kt)
        for s in range(0, NB):
            vt = iopool.tile([128, 2048], F32R, name=f"v_s{s}", tag="vt", bufs=4)
            nc.sync.dma_start(vt[:], mio(v_all, s).bitcast(F32R))
            vts.append(vt)

        rs_4 = ranks(4, mvT_sb4, mvB4, PP=P2)
        n2b_ps = psmall.tile([1, 512], F32, name="n2b_ps", tag="psm", bufs=1)
        for tt in range(4):
            P2(nc.tensor.matmul(n2b_ps[0:1, 128 * tt:128 * (tt + 1)],
                                rs_4[:, tt:tt + 1], idI[:], start=True, stop=True,
                                is_transpose=True))
        n2b_sb = wpool.tile([1, 512], F32, name="n2b_sb", bufs=1)
        P2(nc.scalar.copy(n2b_sb[:], n2b_ps[:]))
        P2(nc.gpsimd.partition_broadcast(n2bB[:], n2b_sb[:]))

        # ---------- local slots: phase A (FFT/mv/ranks) ----------
        mvs = []
        for s in range(0, NB):
            qt, kt = qts[s], kts[s]
            mvT_sb, mv_sb, mvB = stage_mv(s, lambda lc, qt=qt: qt[:, 512 * lc:512 * (lc + 1)],
                                          lambda lc, kt=kt: kt[:, 512 * lc:512 * (lc + 1)])
            rs = ranks(s, mvT_sb, mvB)
            mvs.append((mvT_sb, mv_sb, mvB, rs))

        # ---------- phase B for all slots (rank-match, g row, circulant fetch) ----------
        bres = []
        for s in range(NB):
            mvT_sb, mv_sb, mvB, rs = mvs[s]
            # softmax pieces
            expscr = wpool.tile([1, 512], F32, name=f"expscr_{s}", tag="expscr", bufs=1)
            z_sb = wpool.tile([1, 1], F32, name=f"z_{s}", tag="z", bufs=2)
            P(nc.scalar.activation(expscr[:], mv_sb[:], AF.Exp, accum_out=z_sb[:]))
            rz = wpool.tile([1, 1], F32, name=f"rz_{s}", tag="rz", bufs=2)
            P(nc.vector.reciprocal(rz[:], z_sb[:]))
            smc = wpool.tile([128, 4], F32, name=f"smc_{s}", tag="smc", bufs=2)
            P(nc.scalar.activation(smc[:], mvT_sb[:], AF.Exp))
            smcr = wpool.tile([128, 4], F32R, name=f"smcr_{s}", tag="smcr", bufs=2)
            P(nc.vector.tensor_copy(smcr[:], smc[:]))

            # g row via rank match vs the batch-0 row (both raw is_lt counts)
            g_ps = psmall.tile([1, 512], F32, name=f"gps_{s}", tag="outp", bufs=1)
            for j in range(4):
                wt = wpool.tile([128, 512], F32R, name=f"wt_{s}_{j}", tag=f"wt{j}", bufs=1)
                P(nc.gpsimd.tensor_scalar(wt[:], n2bB[:], rs[:, j:j + 1], None, AL.is_equal))
                P(nc.tensor.matmul(g_ps[:], smcr[:, j:j + 1], wt[:], start=(j == 0), stop=(j == 3)))
            gn = wpool.tile([1, 512], F32, name=f"gn_{s}", tag="gn", bufs=2)
            P(nc.scalar.activation(gn[:], g_ps[:], AF.Copy, bias=0.0, scale=rz[:]))

            # ---------- stage C: circulant via doubled row + Hankel + reversal ----------
            gbuf = dpool.tile([1, 1024], F32, name=f"gbuf_{s}", tag="gbuf", bufs=4)
            gb = gbuf[:].tensor
            P(nc.sync.dma_start(bass.AP(tensor=gb, offset=gbuf[:].offset, ap=[[512, 1], [1, 512]]), gn[:]))
            P(nc.sync.dma_start(bass.AP(tensor=gb, offset=gbuf[:].offset + 512, ap=[[512, 1], [1, 512]]), gn[:]))
            Hk = iopool.tile([128, 512], F32, name=f"H_{s}", tag="Hk", bufs=2)
            P(nc.sync.dma_start(Hk[:], bass.AP(tensor=gb, offset=gbuf[:].offset + 385, ap=[[1, 128], [1, 512]])))
            Tt = wpool.tile([128, 512], F32R, name=f"T_{s}", tag="Tt", bufs=4)
            hap = Hk[:]
            rev = bass.AP(tensor=hap.tensor, offset=hap.offset + 127,
                          ap=[[hap.ap[0][0], 128], [128, 4], [-1, 128]])
            P(nc.gpsimd.tensor_copy(Tt[:], rev))
            bres.append(Tt)

        # ---------- phase C per slot: Toeplitz-block aggregation ----------
        for s in range(NB):
            Tt = bres[s]
            vt = vts[s]
            oAll = wpool.tile([128, 2048], F32, name=f"oAll_{s}", tag="oAll", bufs=2)
            for tt in range(4):
                o_ps = psmall.tile([128, 512], F32, name=f"ops_{s}_{tt}",
                                   tag=("outp" if tt % 2 == 0 else "psm"), bufs=1)
                for ss in range(4):
                    d = (ss - tt) % 4
                    nc.tensor.matmul(o_ps[:], Tt[:, 128 * d:128 * (d + 1)],
                                     vt[:, 512 * ss:512 * (ss + 1)],
                                     start=(ss == 0), stop=(ss == 3))
                osb = oAll[:, 512 * tt:512 * (tt + 1)]
                if tt % 2 == 0:
                    P(nc.scalar.copy(osb, o_ps[:]))
                else:
                    P(nc.vector.tensor_copy(osb, o_ps[:]))
                # per-chunk store: the last chunk's 256KB write is all that
                # remains after the final copy, instead of a 1MB slot write
                P(nc.sync.dma_start(bass.AP(tensor=out_all, offset=s * BPB + 65536 * tt,
                                            ap=[[512, 128], [1, 512]]), osb))

    nc.compile()
    _NC_CACHE = nc
    return nc


def kernel(queries, keys, values):
    q = np.ascontiguousarray(queries, dtype=np.float32).reshape(B, L, C)
    k = np.ascontiguousarray(keys, dtype=np.float32).reshape(B, L, C)
    v = np.ascontiguousarray(values, dtype=np.float32).reshape(B, L, C)
    nc = _build()
    in_maps = []
    for c in range(NCORES):
        sl = slice(NB * c, NB * (c + 1))
        in_maps.append({
            "q_all": np.concatenate([q[sl], q[0:1]], axis=0),
            "k_all": np.concatenate([k[sl], k[0:1]], axis=0),
            "v_all": v[sl],
        })
    res = run_bass_kernel_spmd(nc, in_maps, core_ids=list(range(NCORES)))
    out = np.concatenate([res.results[c]["out_all"] for c in range(NCORES)], axis=0)
    return out.reshape(B, L, H, E)


if __name__ == "__main__":
    rng = np.random.default_rng(0)
    qq = rng.standard_normal((B, L, H, E)).astype(np.float32)
    kk = rng.standard_normal((B, L, H, E)).astype(np.float32)
    vv = rng.standard_normal((B, L, H, E)).astype(np.float32)
    o = kernel(queries=qq, keys=kk, values=vv)
    print(o.shape, o.dtype, np.abs(o).max())
